# revision 15
# baseline (speedup 1.0000x reference)
"""Trainium2 Bass kernel for GQA attention (B=4, T=2048, D=2048, 16 heads / 4 kv groups, RoPE).

Sharding: 8 cores = 4 batches x 2 head-halves. Core c handles batch c//2 and
heads (c%2)*8..+8 with kv groups (c%2)*2..+2.  Per core:
  qkvT projection (channel-major) with RoPE fused into the PSUM eviction,
  two-pass softmax (pass A: S[q,k] row-sums via activation accum_out;
  pass B: S^T[k,q] recomputed by swapping matmul operands, exp, PV in
  natural [q,d] orientation so 1/l is a per-partition scale at eviction),
  PE transpose to d-major, then row-parallel o_proj giving a partial
  [T, D] that the host sums across the two half cores of each batch.
All matmuls in bf16 with fp32 PSUM accumulation.
"""

import numpy as np
import ml_dtypes

BF16 = ml_dtypes.bfloat16

D_MODEL = 2048
NUM_HEADS = 16
QUERY_GROUPS = 4
HEAD_DIM = 128
B = 4
T = 2048
THETA = 10000.0
SCALE = 0.08838834764831845
N_CORES = 8

P = 128
NH = NUM_HEADS // 2          # 8 q heads per core
NG = QUERY_GROUPS // 2       # 2 kv groups per core
QDIM = NH * HEAD_DIM         # 1024
GDIM = NG * HEAD_DIM         # 256
NKT = D_MODEL // P           # 16 contraction tiles over d_model
NTT = T // P                 # 16 tiles over sequence
NCH = T // 512               # 4 chunks of 512 over sequence
NDT = QDIM // P              # 8 head/dim tiles per core


def build_nc(masked: bool):
    import concourse.bacc as bacc
    import concourse.tile as tile
    import concourse.mybir as mybir
    from concourse.masks import make_identity
    from contextlib import ExitStack

    dt = mybir.dt
    f32 = dt.float32
    bf16 = dt.bfloat16
    AF = mybir.ActivationFunctionType

    nc = bacc.Bacc("TRN2", target_bir_lowering=False, debug=False, num_devices=N_CORES)

    xt = nc.dram_tensor("xt", [D_MODEL, T], bf16, kind="ExternalInput")
    wqk = nc.dram_tensor("wqk", [D_MODEL, QDIM + GDIM], bf16, kind="ExternalInput")
    wv = nc.dram_tensor("wv", [D_MODEL, GDIM], bf16, kind="ExternalInput")
    wo = nc.dram_tensor("wo", [QDIM, D_MODEL], bf16, kind="ExternalInput")
    cosq = nc.dram_tensor("cosq", [P, T], f32, kind="ExternalInput")
    sinq = nc.dram_tensor("sinq", [P, T], f32, kind="ExternalInput")
    cosk = nc.dram_tensor("cosk", [P, T], f32, kind="ExternalInput")
    sink = nc.dram_tensor("sink", [P, T], f32, kind="ExternalInput")
    if masked:
        maskcol = nc.dram_tensor("maskcol", [P, NTT], f32, kind="ExternalInput")
    out = nc.dram_tensor("out", [T, D_MODEL], f32, kind="ExternalOutput")

    with tile.TileContext(nc) as tc:
        with ExitStack() as ctx:
            constp = ctx.enter_context(tc.tile_pool(name="const", bufs=1))
            qkT_pool = ctx.enter_context(tc.tile_pool(name="qkT", bufs=NH + NG))
            vnat_pool = ctx.enter_context(tc.tile_pool(name="vnat", bufs=NTT))

            identity = constp.tile([P, P], bf16, tag="identity")
            make_identity(nc, identity[:])
            if masked:
                maskcol_t = constp.tile([P, NTT], f32, tag="maskcol")
                nc.sync.dma_start(out=maskcol_t[:], in_=maskcol[:, :])

            # persistent bf16 tensors
            qkT = [qkT_pool.tile([P, T], bf16, tag="qkT", name=f"qkT{i}") for i in range(NH + NG)]
            # v_aug layout per t-tile: [v_g0 | ones | v_g1 | ones] so that the
            # 129-wide slice for group g is contiguous; the ones column makes
            # the PV matmul accumulate the softmax denominator in psum col 128.
            v_aug = [vnat_pool.tile([P, NG * (P + 1)], bf16, tag="vnat", name=f"vaug{i}")
                     for i in range(NTT)]
            for i in range(NTT):
                for g in range(NG):
                    nc.vector.memset(v_aug[i][:, g * (P + 1) + P:g * (P + 1) + P + 1], 1.0)

            # ---------------- phase 1: qkv projection + rope -------------
            with ExitStack() as ph1:
                tabp = ph1.enter_context(tc.tile_pool(name="tab", bufs=1))
                wqk_pool = ph1.enter_context(tc.tile_pool(name="wqk", bufs=NKT))
                wv_pool = ph1.enter_context(tc.tile_pool(name="wv", bufs=NKT))
                xc_pool = ph1.enter_context(tc.tile_pool(name="xc", bufs=NKT + 2))
                tmp_pool = ph1.enter_context(tc.tile_pool(name="rtmp", bufs=2))
                pj_pool = ph1.enter_context(
                    tc.tile_pool(name="pj", bufs=4, space="PSUM"))

                def load_xc(nch):
                    c0 = nch * 512
                    xc = []
                    for kt in range(NKT):
                        tl = xc_pool.tile([P, 512], bf16, tag="xc", name=f"xc{kt}")
                        nc.sync.dma_start(
                            out=tl[:], in_=xt[kt * P:(kt + 1) * P, c0:c0 + 512])
                        xc.append(tl)
                    return xc

                wv_t = []
                for kt in range(NKT):
                    tl = wv_pool.tile([P, GDIM], bf16, tag="wv", name=f"wvt{kt}")
                    nc.sync.dma_start(out=tl[:], in_=wv[kt * P:(kt + 1) * P, :])
                    wv_t.append(tl)
                xc_next = load_xc(0)
                wqk_t = []
                for kt in range(NKT):
                    tl = wqk_pool.tile([P, QDIM + GDIM], bf16, tag="wqk", name=f"wqkt{kt}")
                    nc.sync.dma_start(out=tl[:], in_=wqk[kt * P:(kt + 1) * P, :])
                    wqk_t.append(tl)
                tabs = {}
                for nm, tsrc in (("cosq", cosq), ("sinq", sinq),
                                 ("cosk", cosk), ("sink", sink)):
                    tl = tabp.tile([P, T], f32, tag=nm, name=nm + "_t")
                    nc.sync.dma_start(out=tl[:], in_=tsrc[:, :])
                    tabs[nm] = tl

                for nch in range(NCH):
                    c0 = nch * 512
                    xc = xc_next
                    if nch + 1 < NCH:
                        xc_next = load_xc(nch + 1)
                    # v projection first: depends only on xc + wv (2.5 MB),
                    # so PE starts before the full wqk lands
                    for tl_i in range(4):
                        tt = nch * 4 + tl_i
                        ps = pj_pool.tile([P, GDIM], f32, tag="pj")
                        for kt in range(NKT):
                            nc.tensor.matmul(
                                ps[:],
                                lhsT=xc[kt][:, tl_i * P:(tl_i + 1) * P],
                                rhs=wv_t[kt][:],
                                start=(kt == 0), stop=(kt == NKT - 1))
                        for g in range(NG):
                            nc.vector.tensor_copy(
                                v_aug[tt][:, g * (P + 1):g * (P + 1) + P],
                                ps[:, g * P:(g + 1) * P])
                    # q/k channel-major projection with fused rope eviction
                    for m in range(NH + NG):
                        ps = pj_pool.tile([P, 512], f32, tag="pj")
                        for kt in range(NKT):
                            nc.tensor.matmul(
                                ps[:],
                                lhsT=wqk_t[kt][:, m * P:(m + 1) * P],
                                rhs=xc[kt][:],
                                start=(kt == 0), stop=(kt == NKT - 1))
                        ct = tabs["cosq"] if m < NH else tabs["cosk"]
                        st = tabs["sinq"] if m < NH else tabs["sink"]
                        t1 = tmp_pool.tile([P, 512], f32, tag="t1")
                        t2 = tmp_pool.tile([P, 512], f32, tag="t2")
                        h = P // 2
                        nc.vector.tensor_mul(t1[:], ps[:], ct[:, c0:c0 + 512])
                        nc.vector.tensor_mul(
                            t2[0:h, :], ps[h:P, :], st[0:h, c0:c0 + 512])
                        nc.vector.tensor_mul(
                            t2[h:P, :], ps[0:h, :], st[h:P, c0:c0 + 512])
                        nc.vector.tensor_add(
                            qkT[m][:, c0:c0 + 512], t1[:], t2[:])

            # phase 2+ pools (opened after phase-1 pools free their SBUF/PSUM)
            mm_pool = ctx.enter_context(tc.tile_pool(name="mm", bufs=4, space="PSUM"))
            pv_pool = ctx.enter_context(tc.tile_pool(name="pv", bufs=4, space="PSUM"))
            attn_pool = ctx.enter_context(tc.tile_pool(name="attn", bufs=NTT))
            aT_pool = ctx.enter_context(tc.tile_pool(name="aT", bufs=NDT))
            rc_pool = ctx.enter_context(tc.tile_pool(name="rc", bufs=8))
            pt_pool = ctx.enter_context(tc.tile_pool(name="pt", bufs=4))
            osb_pool = ctx.enter_context(tc.tile_pool(name="osb", bufs=6))
            wo_pool = ctx.enter_context(tc.tile_pool(name="wo", bufs=NDT))
            attn_t = [attn_pool.tile([P, QDIM], bf16, tag="attn", name=f"attn{i}") for i in range(NTT)]
            aT = [aT_pool.tile([P, T], bf16, tag="aT", name=f"aT{i}") for i in range(NDT)]

            wo_t = []
            for dtile in range(NDT):
                tl = wo_pool.tile([P, D_MODEL], bf16, tag="wo", name=f"wot{dtile}")
                nc.sync.dma_start(out=tl[:], in_=wo[dtile * P:(dtile + 1) * P, :])
                wo_t.append(tl)

            # ---------------- phase 2+3: attention with interleaved o_proj ---
            # qc-outer / head-inner: after each q-chunk, the 4 finished
            # t-tiles are transposed and their o_proj runs, giving PE work
            # that overlaps the ACT-bound exp stretches of the next chunk.
            for qc in range(NCH):
                for h in range(NH):
                    g = h // 4  # local kv group
                    kTg = qkT[NH + g]
                    qTh = qkT[h]
                    pvs = [pv_pool.tile([P, P + 4], f32, tag="pv", name=f"pv{j}")
                           for j in range(4)]
                    for kt in range(NTT):
                        ps = mm_pool.tile([P, 512], f32, tag="mm")
                        nc.tensor.matmul(
                            ps[:],
                            lhsT=kTg[:, kt * P:(kt + 1) * P],
                            rhs=qTh[:, qc * 512:(qc + 1) * 512],
                            start=True, stop=True)
                        pt = pt_pool.tile([P, 512], bf16, tag="pt")
                        if masked:
                            nc.scalar.activation(
                                pt[:], ps[:], AF.Exp, bias=maskcol_t[:, kt:kt + 1])
                        else:
                            nc.scalar.activation(pt[:], ps[:], AF.Exp)
                        for j in range(4):
                            nc.tensor.matmul(
                                pvs[j][:, 0:P + 1],
                                lhsT=pt[:, j * P:(j + 1) * P],
                                rhs=v_aug[kt][:, g * (P + 1):(g + 1) * (P + 1)],
                                start=(kt == 0), stop=(kt == NTT - 1))
                    for j in range(4):
                        qt = qc * 4 + j
                        rc = rc_pool.tile([P, 1], f32, tag="rc")
                        nc.vector.reciprocal(rc[:], pvs[j][:, P:P + 1])
                        nc.vector.tensor_scalar_mul(
                            attn_t[qt][:, h * P:(h + 1) * P],
                            pvs[j][:, 0:P], rc[:])

                # transpose the 4 finished t-tiles to d-major
                for j in range(4):
                    qt = qc * 4 + j
                    for dtile in range(NDT):
                        tps = pv_pool.tile([P, P], bf16, tag="pv")
                        nc.tensor.transpose(
                            tps[:], attn_t[qt][:, dtile * P:(dtile + 1) * P],
                            identity[:])
                        nc.vector.tensor_copy(aT[dtile][:, qt * P:(qt + 1) * P], tps[:])
                # o_proj for the 4 finished t-tiles
                for j in range(4):
                    tt = qc * 4 + j
                    for nchn in range(NCH):
                        ps = mm_pool.tile([P, 512], f32, tag="mm")
                        for dtile in range(NDT):
                            nc.tensor.matmul(
                                ps[:],
                                lhsT=aT[dtile][:, tt * P:(tt + 1) * P],
                                rhs=wo_t[dtile][:, nchn * 512:(nchn + 1) * 512],
                                start=(dtile == 0), stop=(dtile == NDT - 1))
                        osb = osb_pool.tile([P, 512], f32, tag="osb")
                        nc.vector.tensor_copy(osb[:], ps[:])
                        nc.sync.dma_start(
                            out=out[tt * P:(tt + 1) * P, nchn * 512:(nchn + 1) * 512],
                            in_=osb[:])

    nc.compile()
    return nc


def make_tables():
    inv_freq = 1.0 / (THETA ** (np.arange(0, HEAD_DIM, 2, dtype=np.float32)
                                / HEAD_DIM))          # [64]
    ang = np.arange(T, dtype=np.float32)[:, None] * inv_freq[None, :]  # [T, 64]
    cos = np.cos(ang).T.astype(np.float32)            # [64, T]
    sin = np.sin(ang).T.astype(np.float32)
    cos2 = np.concatenate([cos, cos], axis=0)         # [128, T]
    sinA = np.concatenate([-sin, sin], axis=0)        # [128, T]
    return (np.ascontiguousarray(cos2 * SCALE), np.ascontiguousarray(sinA * SCALE),
            np.ascontiguousarray(cos2), np.ascontiguousarray(sinA))


def make_in_maps(x, W_qkv, W_o, padding_mask, masked):
    cosq_v, sinq_v, cosk_v, sink_v = make_tables()
    in_maps = []
    for c in range(N_CORES):
        b, half = c // 2, c % 2
        q0 = half * QDIM
        k0 = NUM_HEADS * HEAD_DIM + half * GDIM
        v0 = NUM_HEADS * HEAD_DIM + QUERY_GROUPS * HEAD_DIM + half * GDIM
        wqk_v = np.concatenate(
            [W_qkv[:, q0:q0 + QDIM], W_qkv[:, k0:k0 + GDIM]], axis=1)
        m = {
            "xt": np.ascontiguousarray(x[b].T).astype(BF16),
            "wqk": np.ascontiguousarray(wqk_v).astype(BF16),
            "wv": np.ascontiguousarray(W_qkv[:, v0:v0 + GDIM]).astype(BF16),
            "wo": np.ascontiguousarray(W_o[half * QDIM:(half + 1) * QDIM, :]).astype(BF16),
            "cosq": cosq_v, "sinq": sinq_v, "cosk": cosk_v, "sink": sink_v,
        }
        if masked:
            bias = np.where(padding_mask[b], 0.0, -1e30).astype(np.float32)  # [T]
            m["maskcol"] = np.ascontiguousarray(
                bias.reshape(NTT, P).T).astype(np.float32)
        in_maps.append(m)
    return in_maps


_nc_cache = {}


def kernel(x, W_qkv, W_o, padding_mask, trace=False):
    from concourse.bass_utils import run_bass_kernel_spmd

    x = np.asarray(x)
    W_qkv = np.asarray(W_qkv)
    W_o = np.asarray(W_o)
    padding_mask = np.asarray(padding_mask)
    masked = not bool(padding_mask.all())

    if masked not in _nc_cache:
        _nc_cache[masked] = build_nc(masked)
    nc = _nc_cache[masked]

    in_maps = make_in_maps(x, W_qkv, W_o, padding_mask, masked)
    res = run_bass_kernel_spmd(
        nc, in_maps, core_ids=list(range(N_CORES)),
        trace=trace, trace_cores=[0] if trace else None)

    out = np.empty((B, T, D_MODEL), np.float32)
    for b in range(B):
        out[b] = res.results[2 * b]["out"] + res.results[2 * b + 1]["out"]
    kernel.last_exec_time_ns = res.exec_time_ns
    kernel.last_results = res
    return out
